# revision 20
# baseline (speedup 1.0000x reference)
"""Causal self-attention with RoPE on 8 TRN2 NeuronCores.

Sharding: 2 (batch) x 4 (head-group tensor parallel). Core c handles
batch b=c//4 and heads [4g, 4g+4) with g=c%4. Each core computes its
q,k,v projections, RoPE, causal attention (transposed-scores flash
layout), and its partial of the output projection; the host sums the
4 partials per batch (the "all-reduce").

v6: one flat software-pipelined schedule. The attention kt loop is
ACT(exp)-bound, so QKV/proj matmuls are pulled from a filler queue
into the PE stream between attention ops; AV/rowsum are delayed one
k-tile behind scores so the PE never waits on exp. Rowsum matmuls are
issued after both AV passes as 4 concurrent M=32 PE tiles (one pass
instead of two) which also initializes the full rowsum PSUM bank so
the reciprocal can run on it directly. The causal mask multiply runs
on GpSimd to keep DVE off the inner-loop critical path.

Self-contained: hardcodes shapes from the problem spec.
"""
import numpy as np
import ml_dtypes

import concourse.bass as bass
import concourse.mybir as mybir
import concourse.tile as tile
from concourse import bacc
from concourse.bass_utils import run_bass_kernel_spmd

F32 = mybir.dt.float32
BF16 = mybir.dt.bfloat16

B, T, DIM = 2, 2048, 1024
HEADS, HEAD_DIM = 16, 64
INNER = HEADS * HEAD_DIM
ROPE_BASE = 10000.0
N_CORES = 8
TPG = 4                      # tensor-parallel group size (head groups)
HPC = HEADS // TPG           # heads per core = 4
LOC = HPC * HEAD_DIM         # local inner = 256
SCALE = 1.0 / np.sqrt(HEAD_DIM)

TB = 512                     # t block for QKV / q block for attention
NTB = T // TB                # 4
ND = DIM // 128              # 8 contraction chunks


def _host_constants():
    inv_freq = 1.0 / (ROPE_BASE ** (np.arange(0, HEAD_DIM, 2, dtype=np.float32) / HEAD_DIM))
    t = np.arange(T, dtype=np.float32)
    freqs = np.outer(t, inv_freq).astype(np.float32)          # [T, 32]
    cos32 = np.cos(freqs).T.astype(np.float32)                # [32, T]
    sin32 = np.sin(freqs).T.astype(np.float32)
    cosT = np.tile(cos32, (4, 1))                             # [128, T]
    sinT = np.tile(sin32, (4, 1))

    # rot matrix: rot[m] = -x[m+32] (m%64<32), +x[m-32] (m%64>=32); lhsT[k, m]
    prot = np.zeros((128, 128), dtype=np.float32)
    for blk in range(2):
        o = blk * 64
        for m in range(32):
            prot[o + m + 32, o + m] = -1.0
            prot[o + m, o + m + 32] = 1.0

    # post-exp 0/1 causal mask for the diagonal 128-col block: keep j >= p
    j = np.arange(128)[None, :]
    p = np.arange(128)[:, None]
    mask01 = (j >= p).astype(ml_dtypes.bfloat16)              # [128, 128]
    return cosT, sinT, prot, mask01


def build_kernel(tc):
    nc = tc.nc
    xT = nc.dram_tensor("xT", [DIM, T], BF16, kind="ExternalInput").ap()
    w_qk = nc.dram_tensor("w_qk", [DIM, 2 * LOC], BF16, kind="ExternalInput").ap()
    w_v = nc.dram_tensor("w_v", [DIM, LOC], BF16, kind="ExternalInput").ap()
    w_pr = nc.dram_tensor("w_pr", [LOC, DIM], BF16, kind="ExternalInput").ap()
    cosT_d = nc.dram_tensor("cosT", [128, T], BF16, kind="ExternalInput").ap()
    sinT_d = nc.dram_tensor("sinT", [128, T], BF16, kind="ExternalInput").ap()
    prot_d = nc.dram_tensor("prot", [128, 128], BF16, kind="ExternalInput").ap()
    mask01_d = nc.dram_tensor("mask01", [128, 128], BF16, kind="ExternalInput").ap()
    out_d = nc.dram_tensor("out", [T, DIM], BF16, kind="ExternalOutput").ap()

    with (
        tc.tile_pool(name="const", bufs=1) as const,
        tc.tile_pool(name="persist", bufs=1) as persist,
        tc.tile_pool(name="work", bufs=2) as work,
        tc.tile_pool(name="expp", bufs=6) as expp,
        tc.tile_pool(name="ps_sc", bufs=2, space="PSUM") as ps_sc,
        tc.tile_pool(name="ps_av", bufs=1, space="PSUM") as ps_av,
        tc.tile_pool(name="ps_mm", bufs=1, space="PSUM") as ps_mm,
    ):
        # ---- input DMAs, latency-ordered: first QKV chunk ASAP ----
        wqk_all = const.tile([128, ND, 2 * LOC], BF16, tag="wqk")
        xt_sb = [const.tile([128, ND, TB], BF16, tag=f"xt{tb}", name=f"xt{tb}")
                 for tb in range(NTB)]
        nc.sync.dma_start(out=wqk_all[:, 0, :], in_=w_qk[0:128, :])
        nc.scalar.dma_start(out=xt_sb[0][:, 0, :], in_=xT[0:128, 0:TB])
        nc.sync.dma_start(
            out=wqk_all[:, 1:ND, :],
            in_=w_qk[128:DIM, :].rearrange("(a p) n -> p a n", p=128))
        nc.scalar.dma_start(
            out=xt_sb[0][:, 1:ND, :],
            in_=xT[128:DIM, 0:TB].rearrange("(a p) t -> p a t", p=128))
        prot_sb = const.tile([128, 128], BF16, tag="prot")
        nc.gpsimd.dma_start(out=prot_sb, in_=prot_d)
        cos_sb = const.tile([128, T], BF16, tag="cos")
        sin_sb = const.tile([128, T], BF16, tag="sin")
        nc.gpsimd.dma_start(out=cos_sb[:, 0:TB], in_=cosT_d[:, 0:TB])
        nc.gpsimd.dma_start(out=sin_sb[:, 0:TB], in_=sinT_d[:, 0:TB])
        wv_all = const.tile([128, ND, LOC], BF16, tag="wv")
        nc.sync.dma_start(
            out=wv_all, in_=w_v.rearrange("(a p) n -> p a n", p=128))
        mask_sb = const.tile([128, 128], BF16, tag="mask")
        nc.sync.dma_start(out=mask_sb, in_=mask01_d)
        nc.sync.dma_start(out=cos_sb[:, TB:T], in_=cosT_d[:, TB:T])
        nc.sync.dma_start(out=sin_sb[:, TB:T], in_=sinT_d[:, TB:T])
        nc.sync.dma_start(
            out=xt_sb[1], in_=xT[:, TB:2 * TB].rearrange("(a p) t -> p a t", p=128))
        wpr_all = const.tile([128, 2, DIM], BF16, tag="wpr")
        nc.sync.dma_start(
            out=wpr_all, in_=w_pr.rearrange("(c p) n -> p c n", p=128))
        nc.sync.dma_start(
            out=xt_sb[2], in_=xT[:, 2 * TB:3 * TB].rearrange("(a p) t -> p a t", p=128))
        nc.sync.dma_start(
            out=xt_sb[3], in_=xT[:, 3 * TB:4 * TB].rearrange("(a p) t -> p a t", p=128))

        ones32 = const.tile([128, 32], BF16, tag="ones32")
        nc.vector.memset(ones32, 1.0)
        ones64f = const.tile([128, 64], F32, tag="ones64f")
        nc.vector.memset(ones64f, 1.0)

        # persistent per-phase outputs
        qk_rope = [[persist.tile([128, TB], BF16, tag=f"qkr{m}_{tb}", name=f"qkr{m}_{tb}")
                    for tb in range(NTB)] for m in range(4)]
        v_sb = [persist.tile([128, LOC], BF16, tag=f"v{ts}", name=f"v{ts}")
                for ts in range(4 * NTB)]
        outT_sb = [[None, None] for _ in range(NTB)]

        mask_bc = mask_sb.rearrange("p (o n) -> p o n", o=1).to_broadcast([128, 2, 128])

        # ---- QKV+RoPE+V production. psum_tags rotates the accumulation
        # bank so chained groups pipeline instead of stalling on the
        # PSUM->SBUF drain of the previous group. ----
        v_issued = set()

        def qkv_gen(tb, psum_banks=((ps_mm, "mm"),), v_part=True):
            csl = slice(tb * TB, (tb + 1) * TB)
            tagc = [0]
            raws = {}

            def next_tile(shape, nm):
                pool, t = psum_banks[tagc[0] % len(psum_banks)]
                tagc[0] += 1
                return pool.tile(shape, F32, tag=t, name=nm)

            def qk_chain(m):
                qk1 = next_tile([128, TB], f"qk1_{m}_{tb}")
                for d in range(ND):
                    nc.tensor.matmul(
                        qk1,
                        lhsT=wqk_all[:, d, m * 128:(m + 1) * 128],
                        rhs=xt_sb[tb][:, d, :],
                        start=(d == 0), stop=(d == ND - 1),
                    )
                    if d in (2, 5):
                        yield 650
                raw = work.tile([128, TB], BF16, tag="raw", name=f"raw_{m}_{tb}")
                nc.vector.tensor_copy(raw, qk1)
                raws[m] = raw
                yield 450

            def rot_chain(m):
                rot = next_tile([128, TB], f"rot_{m}_{tb}")
                nc.tensor.matmul(rot, lhsT=prot_sb, rhs=raws[m], start=True, stop=True)
                qc = work.tile([128, TB], BF16, tag="qc")
                nc.vector.tensor_mul(qc, raws[m], cos_sb[:, csl])
                yield 450
                rs = work.tile([128, TB], BF16, tag="rs")
                nc.vector.tensor_mul(rs, rot, sin_sb[:, csl])
                nc.vector.tensor_add(qk_rope[m][tb], qc, rs)
                yield 400

            def v_chain(s):
                ts = tb * 4 + s
                v_ps = next_tile([128, LOC], f"vps_{ts}")
                for d in range(ND):
                    nc.tensor.matmul(
                        v_ps,
                        lhsT=xt_sb[tb][:, d, s * 128:(s + 1) * 128],
                        rhs=wv_all[:, d, :],
                        start=(d == 0), stop=(d == ND - 1),
                    )
                    if d == 3:
                        yield 550
                nc.vector.tensor_copy(v_sb[ts], v_ps)
                v_issued.add(ts)
                yield 550

            if len(psum_banks) == 1:
                for m in range(4):  # 0,1 -> q pairs; 2,3 -> k pairs
                    yield from qk_chain(m)
                    yield from rot_chain(m)
                for s in range(4):
                    yield from v_chain(s)
            else:
                # software-pipelined: rot(m) issues one chain after qk(m) so
                # the raw PSUM->SBUF copy is done before rot hits the PE queue
                yield from qk_chain(0)
                yield from qk_chain(1)
                yield from rot_chain(0)
                yield from qk_chain(2)
                yield from rot_chain(1)
                yield from qk_chain(3)
                yield from rot_chain(2)
                yield from v_chain(0)
                yield from rot_chain(3)
                if v_part:
                    for s in range(1, 4):
                        yield from v_chain(s)

        def v_gen(tb):
            for s in range(1, 4):
                ts = tb * 4 + s
                v_ps = ps_mm.tile([128, LOC], F32, tag="mm", name=f"vps_{ts}")
                for d in range(ND):
                    nc.tensor.matmul(
                        v_ps,
                        lhsT=xt_sb[tb][:, d, s * 128:(s + 1) * 128],
                        rhs=wv_all[:, d, :],
                        start=(d == 0), stop=(d == ND - 1),
                    )
                    if d == 3:
                        yield 550
                nc.vector.tensor_copy(v_sb[ts], v_ps)
                v_issued.add(ts)
                yield 550

        def proj_gen(qb, psum_banks=((ps_mm, "mm"),)):
            tagc = [0]
            for s in range(4):
                prsb = work.tile([128, 2, TB], BF16, tag="prsb", name=f"prsb_{qb}_{s}")
                for n in range(2):
                    pool, t = psum_banks[tagc[0] % len(psum_banks)]
                    pr = pool.tile([128, TB], F32, tag=t,
                                   name=f"pr_{qb}_{s}_{n}")
                    tagc[0] += 1
                    for p in range(2):
                        nc.tensor.matmul(
                            pr,
                            lhsT=outT_sb[qb][p][:, s * 128:(s + 1) * 128],
                            rhs=wpr_all[:, p, n * TB:(n + 1) * TB],
                            start=(p == 0), stop=(p == 1),
                        )
                    nc.vector.tensor_copy(prsb[:, n, :], pr)
                    yield 700
                row = (qb * 4 + s) * 128
                nc.sync.dma_start(out=out_d[row:row + 128, :],
                                  in_=prsb.rearrange("p a n -> p (a n)"))
                yield 150

        fill = []   # FIFO of (id, generator)

        def pull(budget_ns):
            spent = 0.0
            while fill and spent < budget_ns:
                _, g = fill[0]
                try:
                    spent += next(g)
                except StopIteration:
                    fill.pop(0)
            return spent

        def drain_through(gid):
            while fill and fill[0][0] <= gid:
                _, g = fill[0]
                try:
                    next(g)
                except StopIteration:
                    fill.pop(0)

        # ---- attention for one q block, AV delayed one k tile ----
        def attention(qb):
            nkt = 4 * (qb + 1)
            av_ps = [ps_av.tile([128, TB], F32, tag=f"av{p}", name=f"av{p}_{qb}")
                     for p in range(2)]
            rsum_ps = ps_av.tile([128, TB], F32, tag="rsum", name=f"rsum_{qb}")

            def issue_av(kt, exps, a, w, last):
                st = (kt == 0)
                for p in range(2):
                    for j in range(2):
                        h = 2 * p + j
                        nc.tensor.matmul(
                            av_ps[p][64 * j:64 * j + 64, a:TB],
                            lhsT=v_sb[kt][:, 64 * h:64 * h + 64],
                            rhs=exps[p][:, j, 0:w],
                            start=st, stop=last,
                            skip_group_check=True, tile_position=(0, 64 * j),
                        )
                for p in range(2):
                    for j in range(2):
                        h = 2 * p + j
                        nc.tensor.matmul(
                            rsum_ps[32 * h:32 * h + 32, a:TB],
                            lhsT=ones32,
                            rhs=exps[p][:, j, 0:w],
                            start=st, stop=last,
                            skip_group_check=True, tile_position=(0, 32 * h),
                        )

            def need_v(kt):
                while kt not in v_issued and fill:
                    _, g = fill[0]
                    try:
                        next(g)
                    except StopIteration:
                        fill.pop(0)

            pend = None
            for kt in range(nkt):
                ktl = kt - 4 * qb
                a = 128 * ktl if ktl >= 0 else 0
                w = TB - a
                tbk, ok = kt // 4, (kt % 4) * 128
                exps = []
                for p in range(2):
                    sc2 = ps_sc.tile([128, 2, TB], F32, tag="sc", name=f"sc{qb}_{kt}_{p}")
                    for j in range(2):
                        nc.tensor.matmul(
                            sc2[:, j, 0:w],
                            lhsT=qk_rope[2 + p][tbk][64 * j:64 * j + 64, ok:ok + 128],
                            rhs=qk_rope[p][qb][64 * j:64 * j + 64, a:TB],
                            start=True, stop=True, tile_position=(64 * j, 0),
                        )
                    exp2 = expp.tile([128, 2, TB], BF16, tag="exp", name=f"exp{qb}_{kt}_{p}")
                    nc.scalar.activation(exp2[:, :, 0:w], sc2[:, :, 0:w],
                                         mybir.ActivationFunctionType.Exp,
                                         scale=float(SCALE))
                    if ktl >= 0:
                        nc.gpsimd.tensor_mul(exp2[:, :, 0:128], exp2[:, :, 0:128],
                                             mask_bc)
                    exps.append(exp2)
                # ACT-PE balance: exp ~2*(2w*0.83+430), attn PE ~5w*0.42+500
                budget = 2 * (2 * w * 0.83 + 430) - (5 * w * 0.42 + 500)
                if qb == NTB - 1:
                    # reserve filler to bridge the tail normalizer, where no
                    # next-qb drain exists to keep the PE streaming
                    budget *= 0.75
                pull(0.6 * budget)
                if pend is not None:
                    need_v(pend[0])
                    issue_av(*pend)
                    pull(0.4 * budget)
                pend = (kt, exps, a, w, kt == nkt - 1)
            pull(900)
            need_v(pend[0])
            issue_av(*pend)

            # softmax normalizer: reciprocal on the (fully written) rowsum
            # bank, broadcast via K=1 outer-product matmuls, scale AV
            recip_sb = work.tile([128, TB], F32, tag="recip", name=f"recip_{qb}")
            nc.vector.reciprocal_approx_fast(out=recip_sb, in_=rsum_ps)
            bc_ps = ps_sc.tile([128, 2, TB], F32, tag="sc", name=f"bc{qb}")
            for p in range(2):
                for j in range(2):
                    h = 2 * p + j
                    nc.tensor.matmul(
                        bc_ps[64 * j:64 * j + 64, p, :],
                        lhsT=ones64f[32 * h:32 * h + 1, :],
                        rhs=recip_sb[32 * h:32 * h + 1, :],
                        start=True, stop=True, skip_group_check=True,
                        tile_position=(32 * h, 64 * j),
                    )
            bc_sb = work.tile([128, 2, TB], F32, tag="bcsb", name=f"bcsb_{qb}")
            nc.vector.tensor_copy(bc_sb, bc_ps)
            for p in range(2):
                o_t = persist.tile([128, TB], BF16, tag=f"outT{qb}_{p}",
                                   name=f"outT{qb}_{p}")
                nc.vector.tensor_mul(o_t, av_ps[p], bc_sb[:, p, :])
                outT_sb[qb][p] = o_t

        # ---- flat schedule ----
        head_banks = ((ps_mm, "mm"), (ps_av, "av0"), (ps_av, "av1"))
        for _ in qkv_gen(0, head_banks):    # serial head: qb0 needs it all
            pass
        for tb in range(1, NTB):
            fill.append((tb, qkv_gen(tb)))
        for qb in range(NTB):
            if qb >= 1:
                drain_through(qb)   # qk_rope/v for this q block must exist
            attention(qb)
            if qb < NTB - 1:
                fill.append((10 + qb, proj_gen(qb)))
        tail_banks = ((ps_mm, "mm"), (ps_av, "av0"), (ps_av, "av1"), (ps_av, "rsum"))
        fill.append((13, proj_gen(NTB - 1, tail_banks)))
        while fill:
            pull(1e9)


def shard_inputs(x, w_qkv, w_proj):
    """Full inputs -> list of 8 per-core input maps."""
    cosT, sinT, prot, mask01 = _host_constants()
    x = np.ascontiguousarray(np.asarray(x, dtype=np.float32))
    w_qkv = np.asarray(w_qkv, dtype=np.float32)
    w_proj = np.asarray(w_proj, dtype=np.float32)
    in_maps = []
    for c in range(N_CORES):
        b, g = c // TPG, c % TPG
        xT = np.ascontiguousarray(x[b].T)                     # [DIM, T]
        wq = w_qkv[:, g * LOC:(g + 1) * LOC]
        wk = w_qkv[:, INNER + g * LOC:INNER + (g + 1) * LOC]
        wv = w_qkv[:, 2 * INNER + g * LOC:2 * INNER + (g + 1) * LOC]
        w_qk = np.ascontiguousarray(np.concatenate([wq, wk], axis=1))  # [DIM, 512]
        w_pr = np.ascontiguousarray(w_proj[g * LOC:(g + 1) * LOC, :])  # [256, DIM]
        in_maps.append({
            "xT": xT.astype(ml_dtypes.bfloat16),
            "w_qk": w_qk.astype(ml_dtypes.bfloat16),
            "w_v": np.ascontiguousarray(wv).astype(ml_dtypes.bfloat16),
            "w_pr": w_pr.astype(ml_dtypes.bfloat16),
            "cosT": cosT.astype(ml_dtypes.bfloat16),
            "sinT": sinT.astype(ml_dtypes.bfloat16),
            "prot": prot.astype(ml_dtypes.bfloat16),
            "mask01": mask01,
        })
    return in_maps


_CACHE = {}


def _get_compiled():
    if "nc" not in _CACHE:
        nc = bacc.Bacc("TRN2", target_bir_lowering=False, debug=False,
                       enable_asserts=True, num_devices=N_CORES)
        with tile.TileContext(nc) as tc:
            build_kernel(tc)
        nc.compile()
        _CACHE["nc"] = nc
    return _CACHE["nc"]


def kernel(x, w_qkv, w_proj):
    nc = _get_compiled()
    in_maps = shard_inputs(x, w_qkv, w_proj)
    res = run_bass_kernel_spmd(nc, in_maps, core_ids=list(range(N_CORES)))
    outs = [res.results[c]["out"] for c in range(N_CORES)]
    full = np.stack([
        np.sum([outs[b * TPG + g] for g in range(TPG)], axis=0, dtype=np.float32)
        for b in range(B)
    ])
    return full.astype(np.float32)


# revision 21
# speedup vs baseline: 1.1658x; 1.1658x over previous
"""Causal self-attention with RoPE on 8 TRN2 NeuronCores.

Sharding: 2 (batch) x 4 (head-group tensor parallel). Core c handles
batch b=c//4 and heads [4g, 4g+4) with g=c%4. Each core computes its
q,k,v projections, RoPE, causal attention (transposed-scores flash
layout), and its partial of the output projection; the host sums the
4 partials per batch (the "all-reduce").

v6: one flat software-pipelined schedule. The attention kt loop is
ACT(exp)-bound, so QKV/proj matmuls are pulled from a filler queue
into the PE stream between attention ops; AV/rowsum are delayed one
k-tile behind scores so the PE never waits on exp. Rowsum matmuls are
issued after both AV passes as 4 concurrent M=32 PE tiles (one pass
instead of two) which also initializes the full rowsum PSUM bank so
the reciprocal can run on it directly. The causal mask multiply runs
on GpSimd to keep DVE off the inner-loop critical path.

Self-contained: hardcodes shapes from the problem spec.
"""
import numpy as np
import ml_dtypes

import concourse.bass as bass
import concourse.mybir as mybir
import concourse.tile as tile
from concourse import bacc
from concourse.bass_utils import run_bass_kernel_spmd

F32 = mybir.dt.float32
BF16 = mybir.dt.bfloat16

B, T, DIM = 2, 2048, 1024
HEADS, HEAD_DIM = 16, 64
INNER = HEADS * HEAD_DIM
ROPE_BASE = 10000.0
N_CORES = 8
TPG = 4                      # tensor-parallel group size (head groups)
HPC = HEADS // TPG           # heads per core = 4
LOC = HPC * HEAD_DIM         # local inner = 256
SCALE = 1.0 / np.sqrt(HEAD_DIM)

TB = 512                     # t block for QKV / q block for attention
NTB = T // TB                # 4
ND = DIM // 128              # 8 contraction chunks


def _host_constants():
    inv_freq = 1.0 / (ROPE_BASE ** (np.arange(0, HEAD_DIM, 2, dtype=np.float32) / HEAD_DIM))
    t = np.arange(T, dtype=np.float32)
    freqs = np.outer(t, inv_freq).astype(np.float32)          # [T, 32]
    cos32 = np.cos(freqs).T.astype(np.float32)                # [32, T]
    sin32 = np.sin(freqs).T.astype(np.float32)
    cosT = np.tile(cos32, (4, 1))                             # [128, T]
    sinT = np.tile(sin32, (4, 1))

    # rot matrix: rot[m] = -x[m+32] (m%64<32), +x[m-32] (m%64>=32); lhsT[k, m]
    prot = np.zeros((128, 128), dtype=np.float32)
    for blk in range(2):
        o = blk * 64
        for m in range(32):
            prot[o + m + 32, o + m] = -1.0
            prot[o + m, o + m + 32] = 1.0

    # post-exp 0/1 causal mask for the diagonal 128-col block: keep j >= p
    j = np.arange(128)[None, :]
    p = np.arange(128)[:, None]
    mask01 = (j >= p).astype(ml_dtypes.bfloat16)              # [128, 128]
    return cosT, sinT, prot, mask01


def build_kernel(tc):
    nc = tc.nc
    xT = nc.dram_tensor("xT", [DIM, T], BF16, kind="ExternalInput").ap()
    w_qk = nc.dram_tensor("w_qk", [DIM, 2 * LOC], BF16, kind="ExternalInput").ap()
    w_v = nc.dram_tensor("w_v", [DIM, LOC], BF16, kind="ExternalInput").ap()
    w_pr = nc.dram_tensor("w_pr", [LOC, DIM], BF16, kind="ExternalInput").ap()
    cosT_d = nc.dram_tensor("cosT", [128, T], BF16, kind="ExternalInput").ap()
    sinT_d = nc.dram_tensor("sinT", [128, T], BF16, kind="ExternalInput").ap()
    prot_d = nc.dram_tensor("prot", [128, 128], BF16, kind="ExternalInput").ap()
    mask01_d = nc.dram_tensor("mask01", [128, 128], BF16, kind="ExternalInput").ap()
    out_d = nc.dram_tensor("out", [T, DIM], BF16, kind="ExternalOutput").ap()

    with (
        tc.tile_pool(name="const", bufs=1) as const,
        tc.tile_pool(name="persist", bufs=1) as persist,
        tc.tile_pool(name="work", bufs=2) as work,
        tc.tile_pool(name="expp", bufs=6) as expp,
        tc.tile_pool(name="ps_sc", bufs=2, space="PSUM") as ps_sc,
        tc.tile_pool(name="ps_av", bufs=1, space="PSUM") as ps_av,
        tc.tile_pool(name="ps_mm", bufs=1, space="PSUM") as ps_mm,
    ):
        # ---- input DMAs, latency-ordered: first QKV chunk ASAP ----
        wqk_all = const.tile([128, ND, 2 * LOC], BF16, tag="wqk")
        xt_sb = [const.tile([128, ND, TB], BF16, tag=f"xt{tb}", name=f"xt{tb}")
                 for tb in range(NTB)]
        nc.sync.dma_start(out=wqk_all[:, 0, :], in_=w_qk[0:128, :])
        nc.scalar.dma_start(out=xt_sb[0][:, 0, :], in_=xT[0:128, 0:TB])
        nc.sync.dma_start(
            out=wqk_all[:, 1:ND, :],
            in_=w_qk[128:DIM, :].rearrange("(a p) n -> p a n", p=128))
        nc.scalar.dma_start(
            out=xt_sb[0][:, 1:ND, :],
            in_=xT[128:DIM, 0:TB].rearrange("(a p) t -> p a t", p=128))
        prot_sb = const.tile([128, 128], BF16, tag="prot")
        nc.gpsimd.dma_start(out=prot_sb, in_=prot_d)
        cos_sb = const.tile([128, T], BF16, tag="cos")
        sin_sb = const.tile([128, T], BF16, tag="sin")
        nc.gpsimd.dma_start(out=cos_sb[:, 0:TB], in_=cosT_d[:, 0:TB])
        nc.gpsimd.dma_start(out=sin_sb[:, 0:TB], in_=sinT_d[:, 0:TB])
        wv_all = const.tile([128, ND, LOC], BF16, tag="wv")
        nc.sync.dma_start(
            out=wv_all, in_=w_v.rearrange("(a p) n -> p a n", p=128))
        mask_sb = const.tile([128, 128], BF16, tag="mask")
        nc.sync.dma_start(out=mask_sb, in_=mask01_d)
        nc.sync.dma_start(out=cos_sb[:, TB:T], in_=cosT_d[:, TB:T])
        nc.sync.dma_start(out=sin_sb[:, TB:T], in_=sinT_d[:, TB:T])
        nc.sync.dma_start(
            out=xt_sb[1], in_=xT[:, TB:2 * TB].rearrange("(a p) t -> p a t", p=128))
        wpr_all = const.tile([128, 2, DIM], BF16, tag="wpr")
        nc.sync.dma_start(
            out=wpr_all, in_=w_pr.rearrange("(c p) n -> p c n", p=128))
        nc.sync.dma_start(
            out=xt_sb[2], in_=xT[:, 2 * TB:3 * TB].rearrange("(a p) t -> p a t", p=128))
        nc.sync.dma_start(
            out=xt_sb[3], in_=xT[:, 3 * TB:4 * TB].rearrange("(a p) t -> p a t", p=128))

        ones32 = const.tile([128, 32], BF16, tag="ones32")
        nc.vector.memset(ones32, 1.0)
        ones64f = const.tile([128, 64], F32, tag="ones64f")
        nc.vector.memset(ones64f, 1.0)

        # persistent per-phase outputs
        qk_rope = [[persist.tile([128, TB], BF16, tag=f"qkr{m}_{tb}", name=f"qkr{m}_{tb}")
                    for tb in range(NTB)] for m in range(4)]
        v_sb = [persist.tile([128, LOC], BF16, tag=f"v{ts}", name=f"v{ts}")
                for ts in range(4 * NTB)]
        outT_sb = [[None, None] for _ in range(NTB)]

        mask_bc = mask_sb.rearrange("p (o n) -> p o n", o=1).to_broadcast([128, 2, 128])

        # ---- QKV+RoPE+V production. psum_tags rotates the accumulation
        # bank so chained groups pipeline instead of stalling on the
        # PSUM->SBUF drain of the previous group. ----
        v_issued = set()

        def qkv_gen(tb, psum_banks=((ps_mm, "mm"),), v_part=True):
            csl = slice(tb * TB, (tb + 1) * TB)
            tagc = [0]
            raws = {}

            def next_tile(shape, nm):
                pool, t = psum_banks[tagc[0] % len(psum_banks)]
                tagc[0] += 1
                return pool.tile(shape, F32, tag=t, name=nm)

            def qk_chain(m):
                qk1 = next_tile([128, TB], f"qk1_{m}_{tb}")
                for d in range(ND):
                    nc.tensor.matmul(
                        qk1,
                        lhsT=wqk_all[:, d, m * 128:(m + 1) * 128],
                        rhs=xt_sb[tb][:, d, :],
                        start=(d == 0), stop=(d == ND - 1),
                    )
                    if d in (2, 5):
                        yield 650
                raw = work.tile([128, TB], BF16, tag="raw", name=f"raw_{m}_{tb}")
                nc.vector.tensor_copy(raw, qk1)
                raws[m] = raw
                yield 450

            def rot_chain(m):
                rot = next_tile([128, TB], f"rot_{m}_{tb}")
                nc.tensor.matmul(rot, lhsT=prot_sb, rhs=raws[m], start=True, stop=True)
                qc = work.tile([128, TB], BF16, tag="qc")
                nc.vector.tensor_mul(qc, raws[m], cos_sb[:, csl])
                yield 450
                rs = work.tile([128, TB], BF16, tag="rs")
                nc.vector.tensor_mul(rs, rot, sin_sb[:, csl])
                nc.vector.tensor_add(qk_rope[m][tb], qc, rs)
                yield 400

            def v_chain(s):
                ts = tb * 4 + s
                v_ps = next_tile([128, LOC], f"vps_{ts}")
                for d in range(ND):
                    nc.tensor.matmul(
                        v_ps,
                        lhsT=xt_sb[tb][:, d, s * 128:(s + 1) * 128],
                        rhs=wv_all[:, d, :],
                        start=(d == 0), stop=(d == ND - 1),
                    )
                    if d == 3:
                        yield 550
                nc.vector.tensor_copy(v_sb[ts], v_ps)
                v_issued.add(ts)
                yield 550

            if len(psum_banks) == 1:
                for m in range(4):  # 0,1 -> q pairs; 2,3 -> k pairs
                    yield from qk_chain(m)
                    yield from rot_chain(m)
                for s in range(4):
                    yield from v_chain(s)
            else:
                # software-pipelined: rot(m) issues one chain after qk(m) so
                # the raw PSUM->SBUF copy is done before rot hits the PE queue
                yield from qk_chain(0)
                yield from qk_chain(1)
                yield from rot_chain(0)
                yield from qk_chain(2)
                yield from rot_chain(1)
                yield from qk_chain(3)
                yield from rot_chain(2)
                yield from v_chain(0)
                yield from rot_chain(3)
                if v_part:
                    for s in range(1, 4):
                        yield from v_chain(s)

        def v_gen(tb):
            for s in range(1, 4):
                ts = tb * 4 + s
                v_ps = ps_mm.tile([128, LOC], F32, tag="mm", name=f"vps_{ts}")
                for d in range(ND):
                    nc.tensor.matmul(
                        v_ps,
                        lhsT=xt_sb[tb][:, d, s * 128:(s + 1) * 128],
                        rhs=wv_all[:, d, :],
                        start=(d == 0), stop=(d == ND - 1),
                    )
                    if d == 3:
                        yield 550
                nc.vector.tensor_copy(v_sb[ts], v_ps)
                v_issued.add(ts)
                yield 550

        def proj_gen(qb, psum_banks=((ps_mm, "mm"),)):
            tagc = [0]
            for s in range(4):
                prsb = work.tile([128, 2, TB], BF16, tag="prsb", name=f"prsb_{qb}_{s}")
                for n in range(2):
                    pool, t = psum_banks[tagc[0] % len(psum_banks)]
                    pr = pool.tile([128, TB], F32, tag=t,
                                   name=f"pr_{qb}_{s}_{n}")
                    tagc[0] += 1
                    for p in range(2):
                        nc.tensor.matmul(
                            pr,
                            lhsT=outT_sb[qb][p][:, s * 128:(s + 1) * 128],
                            rhs=wpr_all[:, p, n * TB:(n + 1) * TB],
                            start=(p == 0), stop=(p == 1),
                        )
                    nc.vector.tensor_copy(prsb[:, n, :], pr)
                    yield 700
                row = (qb * 4 + s) * 128
                nc.sync.dma_start(out=out_d[row:row + 128, :],
                                  in_=prsb.rearrange("p a n -> p (a n)"))
                yield 150

        fill = []   # FIFO of (id, generator)

        def pull(budget_ns):
            spent = 0.0
            while fill and spent < budget_ns:
                _, g = fill[0]
                try:
                    spent += next(g)
                except StopIteration:
                    fill.pop(0)
            return spent

        def drain_through(gid):
            while fill and fill[0][0] <= gid:
                _, g = fill[0]
                try:
                    next(g)
                except StopIteration:
                    fill.pop(0)

        # ---- attention for one q block, AV delayed one k tile ----
        def attention(qb):
            nkt = 4 * (qb + 1)
            av_ps = [ps_av.tile([128, TB], F32, tag=f"av{p}", name=f"av{p}_{qb}")
                     for p in range(2)]
            rsum_ps = ps_av.tile([128, TB], F32, tag="rsum", name=f"rsum_{qb}")

            def issue_av(kt, exps, a, w, last):
                st = (kt == 0)
                for p in range(2):
                    for j in range(2):
                        h = 2 * p + j
                        nc.tensor.matmul(
                            av_ps[p][64 * j:64 * j + 64, a:TB],
                            lhsT=v_sb[kt][:, 64 * h:64 * h + 64],
                            rhs=exps[p][:, j, 0:w],
                            start=st, stop=last,
                            skip_group_check=True, tile_position=(0, 64 * j),
                        )
                for p in range(2):
                    for j in range(2):
                        h = 2 * p + j
                        nc.tensor.matmul(
                            rsum_ps[32 * h:32 * h + 32, a:TB],
                            lhsT=ones32,
                            rhs=exps[p][:, j, 0:w],
                            start=st, stop=last,
                            skip_group_check=True, tile_position=(0, 32 * h),
                        )

            def need_v(kt):
                while kt not in v_issued and fill:
                    _, g = fill[0]
                    try:
                        next(g)
                    except StopIteration:
                        fill.pop(0)

            pend = None
            for kt in range(nkt):
                ktl = kt - 4 * qb
                a = 128 * ktl if ktl >= 0 else 0
                w = TB - a
                tbk, ok = kt // 4, (kt % 4) * 128
                exps = []
                for p in range(2):
                    sc2 = ps_sc.tile([128, 2, TB], F32, tag="sc", name=f"sc{qb}_{kt}_{p}")
                    for j in range(2):
                        nc.tensor.matmul(
                            sc2[:, j, 0:w],
                            lhsT=qk_rope[2 + p][tbk][64 * j:64 * j + 64, ok:ok + 128],
                            rhs=qk_rope[p][qb][64 * j:64 * j + 64, a:TB],
                            start=True, stop=True, tile_position=(64 * j, 0),
                        )
                    exp2 = expp.tile([128, 2, TB], BF16, tag="exp", name=f"exp{qb}_{kt}_{p}")
                    nc.scalar.activation(exp2[:, :, 0:w], sc2[:, :, 0:w],
                                         mybir.ActivationFunctionType.Exp,
                                         scale=float(SCALE))
                    if ktl >= 0:
                        nc.gpsimd.tensor_mul(exp2[:, :, 0:128], exp2[:, :, 0:128],
                                             mask_bc)
                    exps.append(exp2)
                # ACT-PE balance: exp ~2*(2w*0.83+430), attn PE ~5w*0.42+500
                budget = 2 * (2 * w * 0.83 + 430) - (5 * w * 0.42 + 500)
                pull(0.6 * budget)
                if pend is not None:
                    need_v(pend[0])
                    issue_av(*pend)
                    pull(0.4 * budget)
                pend = (kt, exps, a, w, kt == nkt - 1)
            pull(900)
            need_v(pend[0])
            issue_av(*pend)

            # softmax normalizer: reciprocal on the (fully written) rowsum
            # bank, broadcast via K=1 outer-product matmuls, scale AV
            recip_sb = work.tile([128, TB], F32, tag="recip", name=f"recip_{qb}")
            nc.vector.reciprocal_approx_fast(out=recip_sb, in_=rsum_ps)
            bc_ps = ps_sc.tile([128, 2, TB], F32, tag="sc", name=f"bc{qb}")
            for p in range(2):
                for j in range(2):
                    h = 2 * p + j
                    nc.tensor.matmul(
                        bc_ps[64 * j:64 * j + 64, p, :],
                        lhsT=ones64f[32 * h:32 * h + 1, :],
                        rhs=recip_sb[32 * h:32 * h + 1, :],
                        start=True, stop=True, skip_group_check=True,
                        tile_position=(32 * h, 64 * j),
                    )
            bc_sb = work.tile([128, 2, TB], F32, tag="bcsb", name=f"bcsb_{qb}")
            nc.vector.tensor_copy(bc_sb, bc_ps)
            for p in range(2):
                o_t = persist.tile([128, TB], BF16, tag=f"outT{qb}_{p}",
                                   name=f"outT{qb}_{p}")
                nc.vector.tensor_mul(o_t, av_ps[p], bc_sb[:, p, :])
                outT_sb[qb][p] = o_t

        # ---- flat schedule ----
        head_banks = ((ps_mm, "mm"), (ps_av, "av0"), (ps_av, "av1"))
        for _ in qkv_gen(0, head_banks):    # serial head: qb0 needs it all
            pass
        for tb in range(1, NTB):
            fill.append((tb, qkv_gen(tb)))
        for qb in range(NTB):
            if qb >= 1:
                drain_through(qb)   # qk_rope/v for this q block must exist
            attention(qb)
            if qb < NTB - 1:
                fill.append((10 + qb, proj_gen(qb)))
        tail_banks = ((ps_mm, "mm"), (ps_av, "av0"), (ps_av, "av1"), (ps_av, "rsum"))
        fill.append((13, proj_gen(NTB - 1, tail_banks)))
        while fill:
            pull(1e9)


def shard_inputs(x, w_qkv, w_proj):
    """Full inputs -> list of 8 per-core input maps."""
    cosT, sinT, prot, mask01 = _host_constants()
    x = np.ascontiguousarray(np.asarray(x, dtype=np.float32))
    w_qkv = np.asarray(w_qkv, dtype=np.float32)
    w_proj = np.asarray(w_proj, dtype=np.float32)
    in_maps = []
    for c in range(N_CORES):
        b, g = c // TPG, c % TPG
        xT = np.ascontiguousarray(x[b].T)                     # [DIM, T]
        wq = w_qkv[:, g * LOC:(g + 1) * LOC]
        wk = w_qkv[:, INNER + g * LOC:INNER + (g + 1) * LOC]
        wv = w_qkv[:, 2 * INNER + g * LOC:2 * INNER + (g + 1) * LOC]
        w_qk = np.ascontiguousarray(np.concatenate([wq, wk], axis=1))  # [DIM, 512]
        w_pr = np.ascontiguousarray(w_proj[g * LOC:(g + 1) * LOC, :])  # [256, DIM]
        in_maps.append({
            "xT": xT.astype(ml_dtypes.bfloat16),
            "w_qk": w_qk.astype(ml_dtypes.bfloat16),
            "w_v": np.ascontiguousarray(wv).astype(ml_dtypes.bfloat16),
            "w_pr": w_pr.astype(ml_dtypes.bfloat16),
            "cosT": cosT.astype(ml_dtypes.bfloat16),
            "sinT": sinT.astype(ml_dtypes.bfloat16),
            "prot": prot.astype(ml_dtypes.bfloat16),
            "mask01": mask01,
        })
    return in_maps


_CACHE = {}


def _get_compiled():
    if "nc" not in _CACHE:
        nc = bacc.Bacc("TRN2", target_bir_lowering=False, debug=False,
                       enable_asserts=True, num_devices=N_CORES)
        with tile.TileContext(nc) as tc:
            build_kernel(tc)
        nc.compile()
        _CACHE["nc"] = nc
    return _CACHE["nc"]


def kernel(x, w_qkv, w_proj):
    nc = _get_compiled()
    in_maps = shard_inputs(x, w_qkv, w_proj)
    res = run_bass_kernel_spmd(nc, in_maps, core_ids=list(range(N_CORES)))
    outs = [res.results[c]["out"] for c in range(N_CORES)]
    full = np.stack([
        np.sum([outs[b * TPG + g] for g in range(TPG)], axis=0, dtype=np.float32)
        for b in range(B)
    ])
    return full.astype(np.float32)


# revision 22
# speedup vs baseline: 1.2213x; 1.0476x over previous
"""Causal self-attention with RoPE on 8 TRN2 NeuronCores.

Sharding: 2 (batch) x 4 (head-group tensor parallel). Core c handles
batch b=c//4 and heads [4g, 4g+4) with g=c%4. Each core computes its
q,k,v projections, RoPE, causal attention (transposed-scores flash
layout), and its partial of the output projection; the host sums the
4 partials per batch (the "all-reduce").

v6: one flat software-pipelined schedule. The attention kt loop is
ACT(exp)-bound, so QKV/proj matmuls are pulled from a filler queue
into the PE stream between attention ops; AV/rowsum are delayed one
k-tile behind scores so the PE never waits on exp. Rowsum matmuls are
issued after both AV passes as 4 concurrent M=32 PE tiles (one pass
instead of two) which also initializes the full rowsum PSUM bank so
the reciprocal can run on it directly. The causal mask multiply runs
on GpSimd to keep DVE off the inner-loop critical path.

Self-contained: hardcodes shapes from the problem spec.
"""
import numpy as np
import ml_dtypes

import concourse.bass as bass
import concourse.mybir as mybir
import concourse.tile as tile
from concourse import bacc
from concourse.bass_utils import run_bass_kernel_spmd

F32 = mybir.dt.float32
BF16 = mybir.dt.bfloat16

B, T, DIM = 2, 2048, 1024
HEADS, HEAD_DIM = 16, 64
INNER = HEADS * HEAD_DIM
ROPE_BASE = 10000.0
N_CORES = 8
TPG = 4                      # tensor-parallel group size (head groups)
HPC = HEADS // TPG           # heads per core = 4
LOC = HPC * HEAD_DIM         # local inner = 256
SCALE = 1.0 / np.sqrt(HEAD_DIM)

TB = 512                     # t block for QKV / q block for attention
NTB = T // TB                # 4
ND = DIM // 128              # 8 contraction chunks


# head-dim permutation making rotate_half quadrant-local: pairs (d, d+32)
# land in the same 32-partition quadrant, so DVE stream_shuffle can do the
# rotate (sign folded into the sin table). q.k is invariant since both q
# and k get the same permutation.
PERM64 = np.array(list(range(0, 16)) + list(range(32, 48))
                  + list(range(16, 32)) + list(range(48, 64)))
SHUF_MASK = list(range(16, 32)) + list(range(16))


def _host_constants():
    inv_freq = 1.0 / (ROPE_BASE ** (np.arange(0, HEAD_DIM, 2, dtype=np.float32) / HEAD_DIM))
    t = np.arange(T, dtype=np.float32)
    freqs = np.outer(t, inv_freq).astype(np.float32)          # [T, 32]
    cos32 = np.cos(freqs).T.astype(np.float32)                # [32, T]
    sin32 = np.sin(freqs).T.astype(np.float32)
    cos64 = np.concatenate([cos32, cos32], 0)                 # [64, T]
    sin64 = np.concatenate([sin32, sin32], 0)
    sign = np.where(PERM64 < 32, -1.0, 1.0)[:, None].astype(np.float32)
    cosT = np.tile(cos64[PERM64], (2, 1))                     # [128, T]
    sinT = np.tile(sin64[PERM64] * sign, (2, 1))

    # post-exp 0/1 causal mask for the diagonal 128-col block: keep j >= p
    j = np.arange(128)[None, :]
    p = np.arange(128)[:, None]
    mask01 = (j >= p).astype(ml_dtypes.bfloat16)              # [128, 128]
    return cosT, sinT, mask01


def build_kernel(tc):
    nc = tc.nc
    xT = nc.dram_tensor("xT", [DIM, T], BF16, kind="ExternalInput").ap()
    w_qk = nc.dram_tensor("w_qk", [DIM, 2 * LOC], BF16, kind="ExternalInput").ap()
    w_v = nc.dram_tensor("w_v", [DIM, LOC], BF16, kind="ExternalInput").ap()
    w_pr = nc.dram_tensor("w_pr", [LOC, DIM], BF16, kind="ExternalInput").ap()
    cosT_d = nc.dram_tensor("cosT", [128, T], BF16, kind="ExternalInput").ap()
    sinT_d = nc.dram_tensor("sinT", [128, T], BF16, kind="ExternalInput").ap()
    mask01_d = nc.dram_tensor("mask01", [128, 128], BF16, kind="ExternalInput").ap()
    out_d = nc.dram_tensor("out", [T, DIM], BF16, kind="ExternalOutput").ap()

    with (
        tc.tile_pool(name="const", bufs=1) as const,
        tc.tile_pool(name="persist", bufs=1) as persist,
        tc.tile_pool(name="work", bufs=2) as work,
        tc.tile_pool(name="expp", bufs=6) as expp,
        tc.tile_pool(name="ps_sc", bufs=2, space="PSUM") as ps_sc,
        tc.tile_pool(name="ps_av", bufs=1, space="PSUM") as ps_av,
        tc.tile_pool(name="ps_mm", bufs=1, space="PSUM") as ps_mm,
    ):
        # ---- input DMAs, latency-ordered: first QKV chunk ASAP ----
        wqk_all = const.tile([128, ND, 2 * LOC], BF16, tag="wqk")
        xt_sb = [const.tile([128, ND, TB], BF16, tag=f"xt{tb}", name=f"xt{tb}")
                 for tb in range(NTB)]
        nc.sync.dma_start(out=wqk_all[:, 0, :], in_=w_qk[0:128, :])
        nc.scalar.dma_start(out=xt_sb[0][:, 0, :], in_=xT[0:128, 0:TB])
        nc.sync.dma_start(
            out=wqk_all[:, 1:ND, :],
            in_=w_qk[128:DIM, :].rearrange("(a p) n -> p a n", p=128))
        nc.scalar.dma_start(
            out=xt_sb[0][:, 1:ND, :],
            in_=xT[128:DIM, 0:TB].rearrange("(a p) t -> p a t", p=128))
        cos_sb = const.tile([128, T], BF16, tag="cos")
        sin_sb = const.tile([128, T], BF16, tag="sin")
        nc.gpsimd.dma_start(out=cos_sb[:, 0:TB], in_=cosT_d[:, 0:TB])
        nc.gpsimd.dma_start(out=sin_sb[:, 0:TB], in_=sinT_d[:, 0:TB])
        wv_all = const.tile([128, ND, LOC], BF16, tag="wv")
        nc.sync.dma_start(
            out=wv_all, in_=w_v.rearrange("(a p) n -> p a n", p=128))
        mask_sb = const.tile([128, 128], BF16, tag="mask")
        nc.sync.dma_start(out=mask_sb, in_=mask01_d)
        nc.sync.dma_start(out=cos_sb[:, TB:T], in_=cosT_d[:, TB:T])
        nc.sync.dma_start(out=sin_sb[:, TB:T], in_=sinT_d[:, TB:T])
        nc.sync.dma_start(
            out=xt_sb[1], in_=xT[:, TB:2 * TB].rearrange("(a p) t -> p a t", p=128))
        wpr_all = const.tile([128, 2, DIM], BF16, tag="wpr")
        nc.sync.dma_start(
            out=wpr_all, in_=w_pr.rearrange("(c p) n -> p c n", p=128))
        nc.sync.dma_start(
            out=xt_sb[2], in_=xT[:, 2 * TB:3 * TB].rearrange("(a p) t -> p a t", p=128))
        nc.sync.dma_start(
            out=xt_sb[3], in_=xT[:, 3 * TB:4 * TB].rearrange("(a p) t -> p a t", p=128))

        ones32 = const.tile([128, 32], BF16, tag="ones32")
        nc.vector.memset(ones32, 1.0)
        ones64f = const.tile([128, 64], F32, tag="ones64f")
        nc.vector.memset(ones64f, 1.0)

        # persistent per-phase outputs
        qk_rope = [[persist.tile([128, TB], BF16, tag=f"qkr{m}_{tb}", name=f"qkr{m}_{tb}")
                    for tb in range(NTB)] for m in range(4)]
        v_sb = [persist.tile([128, LOC], BF16, tag=f"v{ts}", name=f"v{ts}")
                for ts in range(4 * NTB)]
        outT_sb = [[None, None] for _ in range(NTB)]

        mask_bc = mask_sb.rearrange("p (o n) -> p o n", o=1).to_broadcast([128, 2, 128])

        # ---- QKV+RoPE+V production. psum_tags rotates the accumulation
        # bank so chained groups pipeline instead of stalling on the
        # PSUM->SBUF drain of the previous group. ----
        v_issued = set()

        def qkv_gen(tb, psum_banks=((ps_mm, "mm"),), v_part=True):
            csl = slice(tb * TB, (tb + 1) * TB)
            tagc = [0]
            raws = {}

            def next_tile(shape, nm):
                pool, t = psum_banks[tagc[0] % len(psum_banks)]
                tagc[0] += 1
                return pool.tile(shape, F32, tag=t, name=nm)

            def qk_chain(m):
                qk1 = next_tile([128, TB], f"qk1_{m}_{tb}")
                for d in range(ND):
                    nc.tensor.matmul(
                        qk1,
                        lhsT=wqk_all[:, d, m * 128:(m + 1) * 128],
                        rhs=xt_sb[tb][:, d, :],
                        start=(d == 0), stop=(d == ND - 1),
                    )
                    if d in (2, 5):
                        yield 650
                raw = work.tile([128, TB], BF16, tag="raw", name=f"raw_{m}_{tb}")
                nc.vector.tensor_copy(raw, qk1)
                raws[m] = raw
                yield 450

            def rot_chain(m):
                rot = work.tile([128, TB], BF16, tag="rot", name=f"rot_{m}_{tb}")
                nc.vector.stream_shuffle(rot, raws[m], SHUF_MASK)
                qc = work.tile([128, TB], BF16, tag="qc")
                nc.vector.tensor_mul(qc, raws[m], cos_sb[:, csl])
                yield 450
                rs = work.tile([128, TB], BF16, tag="rs")
                nc.vector.tensor_mul(rs, rot, sin_sb[:, csl])
                nc.vector.tensor_add(qk_rope[m][tb], qc, rs)
                yield 400

            def v_chain(s):
                ts = tb * 4 + s
                v_ps = next_tile([128, LOC], f"vps_{ts}")
                for d in range(ND):
                    nc.tensor.matmul(
                        v_ps,
                        lhsT=xt_sb[tb][:, d, s * 128:(s + 1) * 128],
                        rhs=wv_all[:, d, :],
                        start=(d == 0), stop=(d == ND - 1),
                    )
                    if d == 3:
                        yield 550
                nc.vector.tensor_copy(v_sb[ts], v_ps)
                v_issued.add(ts)
                yield 550

            if len(psum_banks) == 1:
                for m in range(4):  # 0,1 -> q pairs; 2,3 -> k pairs
                    yield from qk_chain(m)
                    yield from rot_chain(m)
                for s in range(4):
                    yield from v_chain(s)
            else:
                # software-pipelined: rot(m) issues one chain after qk(m) so
                # the raw PSUM->SBUF copy is done before rot hits the PE queue
                yield from qk_chain(0)
                yield from qk_chain(1)
                yield from rot_chain(0)
                yield from qk_chain(2)
                yield from rot_chain(1)
                yield from qk_chain(3)
                yield from rot_chain(2)
                yield from v_chain(0)
                yield from rot_chain(3)
                if v_part:
                    for s in range(1, 4):
                        yield from v_chain(s)

        def v_gen(tb):
            for s in range(1, 4):
                ts = tb * 4 + s
                v_ps = ps_mm.tile([128, LOC], F32, tag="mm", name=f"vps_{ts}")
                for d in range(ND):
                    nc.tensor.matmul(
                        v_ps,
                        lhsT=xt_sb[tb][:, d, s * 128:(s + 1) * 128],
                        rhs=wv_all[:, d, :],
                        start=(d == 0), stop=(d == ND - 1),
                    )
                    if d == 3:
                        yield 550
                nc.vector.tensor_copy(v_sb[ts], v_ps)
                v_issued.add(ts)
                yield 550

        def proj_gen(qb, psum_banks=((ps_mm, "mm"),)):
            tagc = [0]
            for s in range(4):
                prsb = work.tile([128, 2, TB], BF16, tag="prsb", name=f"prsb_{qb}_{s}")
                for n in range(2):
                    pool, t = psum_banks[tagc[0] % len(psum_banks)]
                    pr = pool.tile([128, TB], F32, tag=t,
                                   name=f"pr_{qb}_{s}_{n}")
                    tagc[0] += 1
                    for p in range(2):
                        nc.tensor.matmul(
                            pr,
                            lhsT=outT_sb[qb][p][:, s * 128:(s + 1) * 128],
                            rhs=wpr_all[:, p, n * TB:(n + 1) * TB],
                            start=(p == 0), stop=(p == 1),
                        )
                    nc.vector.tensor_copy(prsb[:, n, :], pr)
                    yield 700
                row = (qb * 4 + s) * 128
                nc.sync.dma_start(out=out_d[row:row + 128, :],
                                  in_=prsb.rearrange("p a n -> p (a n)"))
                yield 150

        fill = []   # FIFO of (id, generator)

        def pull(budget_ns):
            spent = 0.0
            while fill and spent < budget_ns:
                _, g = fill[0]
                try:
                    spent += next(g)
                except StopIteration:
                    fill.pop(0)
            return spent

        def drain_through(gid):
            while fill and fill[0][0] <= gid:
                _, g = fill[0]
                try:
                    next(g)
                except StopIteration:
                    fill.pop(0)

        # ---- attention for one q block, AV delayed one k tile ----
        def attention(qb):
            nkt = 4 * (qb + 1)
            av_ps = [ps_av.tile([128, TB], F32, tag=f"av{p}", name=f"av{p}_{qb}")
                     for p in range(2)]
            rsum_ps = ps_av.tile([128, TB], F32, tag="rsum", name=f"rsum_{qb}")

            def issue_av(kt, exps, a, w, last):
                st = (kt == 0)
                for p in range(2):
                    for j in range(2):
                        h = 2 * p + j
                        nc.tensor.matmul(
                            av_ps[p][64 * j:64 * j + 64, a:TB],
                            lhsT=v_sb[kt][:, 64 * h:64 * h + 64],
                            rhs=exps[p][:, j, 0:w],
                            start=st, stop=last,
                            skip_group_check=True, tile_position=(0, 64 * j),
                        )
                for p in range(2):
                    for j in range(2):
                        h = 2 * p + j
                        nc.tensor.matmul(
                            rsum_ps[32 * h:32 * h + 32, a:TB],
                            lhsT=ones32,
                            rhs=exps[p][:, j, 0:w],
                            start=st, stop=last,
                            skip_group_check=True, tile_position=(0, 32 * h),
                        )

            def need_v(kt):
                while kt not in v_issued and fill:
                    _, g = fill[0]
                    try:
                        next(g)
                    except StopIteration:
                        fill.pop(0)

            pend = None
            for kt in range(nkt):
                ktl = kt - 4 * qb
                a = 128 * ktl if ktl >= 0 else 0
                w = TB - a
                tbk, ok = kt // 4, (kt % 4) * 128
                exps = []
                for p in range(2):
                    sc2 = ps_sc.tile([128, 2, TB], F32, tag="sc", name=f"sc{qb}_{kt}_{p}")
                    for j in range(2):
                        nc.tensor.matmul(
                            sc2[:, j, 0:w],
                            lhsT=qk_rope[2 + p][tbk][64 * j:64 * j + 64, ok:ok + 128],
                            rhs=qk_rope[p][qb][64 * j:64 * j + 64, a:TB],
                            start=True, stop=True, tile_position=(64 * j, 0),
                        )
                    exp2 = expp.tile([128, 2, TB], BF16, tag="exp", name=f"exp{qb}_{kt}_{p}")
                    nc.scalar.activation(exp2[:, :, 0:w], sc2[:, :, 0:w],
                                         mybir.ActivationFunctionType.Exp,
                                         scale=float(SCALE))
                    if ktl >= 0:
                        nc.gpsimd.tensor_mul(exp2[:, :, 0:128], exp2[:, :, 0:128],
                                             mask_bc)
                    exps.append(exp2)
                # ACT-PE balance: exp ~2*(2w*0.83+430), attn PE ~5w*0.42+500
                budget = 2 * (2 * w * 0.83 + 430) - (5 * w * 0.42 + 500)
                pull(0.6 * budget)
                if pend is not None:
                    need_v(pend[0])
                    issue_av(*pend)
                    pull(0.4 * budget)
                pend = (kt, exps, a, w, kt == nkt - 1)
            pull(900)
            need_v(pend[0])
            issue_av(*pend)

            # softmax normalizer: reciprocal on the (fully written) rowsum
            # bank, broadcast via K=1 outer-product matmuls, scale AV
            recip_sb = work.tile([128, TB], F32, tag="recip", name=f"recip_{qb}")
            nc.vector.reciprocal_approx_fast(out=recip_sb, in_=rsum_ps)
            bc_ps = ps_sc.tile([128, 2, TB], F32, tag="sc", name=f"bc{qb}")
            for p in range(2):
                for j in range(2):
                    h = 2 * p + j
                    nc.tensor.matmul(
                        bc_ps[64 * j:64 * j + 64, p, :],
                        lhsT=ones64f[32 * h:32 * h + 1, :],
                        rhs=recip_sb[32 * h:32 * h + 1, :],
                        start=True, stop=True, skip_group_check=True,
                        tile_position=(32 * h, 64 * j),
                    )
            bc_sb = work.tile([128, 2, TB], F32, tag="bcsb", name=f"bcsb_{qb}")
            nc.vector.tensor_copy(bc_sb, bc_ps)
            for p in range(2):
                o_t = persist.tile([128, TB], BF16, tag=f"outT{qb}_{p}",
                                   name=f"outT{qb}_{p}")
                nc.vector.tensor_mul(o_t, av_ps[p], bc_sb[:, p, :])
                outT_sb[qb][p] = o_t

        # ---- flat schedule ----
        head_banks = ((ps_mm, "mm"), (ps_av, "av0"), (ps_av, "av1"))
        for _ in qkv_gen(0, head_banks):    # serial head: qb0 needs it all
            pass
        for tb in range(1, NTB):
            fill.append((tb, qkv_gen(tb)))
        for qb in range(NTB):
            if qb >= 1:
                drain_through(qb)   # qk_rope/v for this q block must exist
            attention(qb)
            if qb < NTB - 1:
                fill.append((10 + qb, proj_gen(qb)))
        tail_banks = ((ps_mm, "mm"), (ps_av, "av0"), (ps_av, "av1"), (ps_av, "rsum"))
        fill.append((13, proj_gen(NTB - 1, tail_banks)))
        while fill:
            pull(1e9)


def shard_inputs(x, w_qkv, w_proj):
    """Full inputs -> list of 8 per-core input maps."""
    cosT, sinT, mask01 = _host_constants()
    x = np.ascontiguousarray(np.asarray(x, dtype=np.float32))
    w_qkv = np.asarray(w_qkv, dtype=np.float32)
    w_proj = np.asarray(w_proj, dtype=np.float32)
    in_maps = []
    for c in range(N_CORES):
        b, g = c // TPG, c % TPG
        xT = np.ascontiguousarray(x[b].T)                     # [DIM, T]
        wq = (w_qkv[:, g * LOC:(g + 1) * LOC]
              .reshape(DIM, HPC, HEAD_DIM)[:, :, PERM64].reshape(DIM, LOC))
        wk = (w_qkv[:, INNER + g * LOC:INNER + (g + 1) * LOC]
              .reshape(DIM, HPC, HEAD_DIM)[:, :, PERM64].reshape(DIM, LOC))
        wv = w_qkv[:, 2 * INNER + g * LOC:2 * INNER + (g + 1) * LOC]
        w_qk = np.ascontiguousarray(np.concatenate([wq, wk], axis=1))  # [DIM, 512]
        w_pr = np.ascontiguousarray(w_proj[g * LOC:(g + 1) * LOC, :])  # [256, DIM]
        in_maps.append({
            "xT": xT.astype(ml_dtypes.bfloat16),
            "w_qk": w_qk.astype(ml_dtypes.bfloat16),
            "w_v": np.ascontiguousarray(wv).astype(ml_dtypes.bfloat16),
            "w_pr": w_pr.astype(ml_dtypes.bfloat16),
            "cosT": cosT.astype(ml_dtypes.bfloat16),
            "sinT": sinT.astype(ml_dtypes.bfloat16),
            "mask01": mask01,
        })
    return in_maps


_CACHE = {}


def _get_compiled():
    if "nc" not in _CACHE:
        nc = bacc.Bacc("TRN2", target_bir_lowering=False, debug=False,
                       enable_asserts=True, num_devices=N_CORES)
        with tile.TileContext(nc) as tc:
            build_kernel(tc)
        nc.compile()
        _CACHE["nc"] = nc
    return _CACHE["nc"]


def kernel(x, w_qkv, w_proj):
    nc = _get_compiled()
    in_maps = shard_inputs(x, w_qkv, w_proj)
    res = run_bass_kernel_spmd(nc, in_maps, core_ids=list(range(N_CORES)))
    outs = [res.results[c]["out"] for c in range(N_CORES)]
    full = np.stack([
        np.sum([outs[b * TPG + g] for g in range(TPG)], axis=0, dtype=np.float32)
        for b in range(B)
    ])
    return full.astype(np.float32)
